# revision 21
# baseline (speedup 1.0000x reference)
"""MoE FFN (top-2, E=8) Trainium2 kernel.

Strategy (expert parallelism, per sharding hint):
  - Router (tiny: [T,D]@[D,E] + softmax/top2) computed on host in fp32,
    replicated logic; aux losses also on host (negligible FLOPs).
  - Tokens are "all-to-all"-ed on host: each of the 8 cores owns one
    expert and receives exactly the tokens routed to it (padded to a
    shared capacity C so the SPMD NEFF has static shapes).
  - Each core runs the dense SwiGLU FFN for its expert in bf16:
        h = silu(X @ Wg) * (X @ Wu);  Y = (h @ Wd) * combine[:, None]
  - Host scatter-adds the two expert contributions per token.

Device layouts (per core; all DMA slabs contiguous):
  xt: [NCH, 8, 128, 512] bf16  -- X^T per (chunk, d-k-tile), 512-token slabs
  wg: [4, 8, 128, 512]   bf16  -- Wg[e] per (f-group, d-k-tile)
  wu: [4, 8, 128, 512]   bf16
  wd: [16, 128, 1024]    bf16  -- Wd[e] per f-k-tile
  cw: [128, NT]          f32   -- combine weights, token-tile columns
  y:  [NT, 2, 128, 512]  f32   -- output slabs (already combine-scaled)

Matmuls (out = lhsT.T @ rhs, contraction on partitions):
  H^T tile [128f, tw] = sum_kt Wg[kt, :, fsl].T @ X^T[kt, :, tslice]
  Y  tile [128t, 512d] = sum_kt hT[kt][:, tslice].T @ Wd[kt, :, dslice]
so no on-device transposes are needed anywhere.
"""

import numpy as np
import ml_dtypes

import concourse.bacc as bacc
import concourse.mybir as mybir
from concourse import tile

B, S, D, F, E = 4, 2048, 1024, 2048, 8
TOP_K = 2
LB_WEIGHT = 0.01
Z_WEIGHT = 0.001
N_CORES = 8

BF16 = mybir.dt.bfloat16
F32 = mybir.dt.float32

_NC_CACHE = {}


def _chunks_for(C):
    r = C % 512
    widths = [512] * (C // 512) + ([r] if r else [])
    chunks = []
    start = 0
    for tw in widths:
        chunks.append((start, tw))
        start += tw
    return chunks


def _build_nc(C: int, repeat: int = 1):
    """Build the per-core Bass module for capacity C (multiple of 128).

    repeat>1 re-runs the compute loop (bench-only, for differential timing).
    """
    NT = C // 128
    chunks = _chunks_for(C)
    NCH = len(chunks)

    nc = bacc.Bacc(
        "TRN2", target_bir_lowering=False, debug=False, num_devices=N_CORES
    )
    xt = nc.dram_tensor("xt", [NCH, 8, 128, 512], BF16, kind="ExternalInput")
    wg = nc.dram_tensor("wg", [4, 8, 128, 512], BF16, kind="ExternalInput")
    wu = nc.dram_tensor("wu", [4, 8, 128, 512], BF16, kind="ExternalInput")
    wd = nc.dram_tensor("wd", [16, 128, 1024], BF16, kind="ExternalInput")
    cw = nc.dram_tensor("cw", [128, NT], F32, kind="ExternalInput")
    y = nc.dram_tensor("y", [NT, 2, 128, 512], F32, kind="ExternalOutput")

    with tile.TileContext(nc) as tc:
        with (
            tc.tile_pool(name="wpool", bufs=1) as wpool,
            tc.tile_pool(name="xpool", bufs=4) as xpool,
            tc.tile_pool(name="hpool", bufs=3) as hpool,
            tc.tile_pool(name="spool", bufs=4) as spool,
            tc.tile_pool(name="opool", bufs=4) as opool,
            tc.tile_pool(name="psum", bufs=3, space="PSUM") as psum,
            tc.tile_pool(name="psumy", bufs=2, space="PSUM") as psumy,
        ):
            # resident weights + combine weights
            wg_sb = wpool.tile([128, 8, F], BF16, tag="wg")
            wu_sb = wpool.tile([128, 8, F], BF16, tag="wu")
            wd_sb = wpool.tile([128, 16, D], BF16, tag="wd")
            cw_sb = wpool.tile([128, NT], F32, tag="cw")

            # chunk-0 activations first (PE's first matmul needs them), then
            # Wg/Wu streamed in 512-wide f-groups so H(ft=0) starts after
            # ~2MB of DMA instead of the full 12MB weight load.
            xt0_sb = xpool.tile([128, 8, 512], BF16, tag="xt")
            for kt in range(8):
                nc.sync.dma_start(xt0_sb[:, kt, :], xt[0, kt])
            for fg in range(4):
                fgs = slice(fg * 512, (fg + 1) * 512)
                for kt in range(8):
                    nc.sync.dma_start(wg_sb[:, kt, fgs], wg[fg, kt])
                for kt in range(8):
                    nc.sync.dma_start(wu_sb[:, kt, fgs], wu[fg, kt])
            for kt in range(16):
                nc.sync.dma_start(wd_sb[:, kt, :], wd[kt])
            nc.sync.dma_start(cw_sb[:], cw[:])

            def body_all(first=False):
                for ci, (start, tw) in enumerate(chunks):
                    body_chunk(ci, start, tw, xt0_sb if (first and ci == 0) else None)

            def body_chunk(ci, start, tw, preloaded=None):
                if preloaded is not None:
                    xt_sb = preloaded
                else:
                    # load X^T chunk: 8 k-tiles of [128, tw]
                    xt_sb = xpool.tile([128, 8, 512], BF16, tag="xt")
                    for kt in range(8):
                        nc.sync.dma_start(xt_sb[:, kt, :tw], xt[ci, kt][:, :tw])

                # H^T/U^T -> silu(H)*U, 16 f-tiles of [128, tw]
                ht_sb = hpool.tile([128, 16, tw], BF16, tag="ht")
                for ft in range(16):
                    ph = psum.tile([128, tw], F32, tag="ph")
                    pu = psum.tile([128, tw], F32, tag="pu")
                    fsl = slice(ft * 128, (ft + 1) * 128)
                    for kt in range(8):
                        nc.tensor.matmul(
                            ph[:],
                            wg_sb[:, kt, fsl],
                            xt_sb[:, kt, :tw],
                            start=(kt == 0),
                            stop=(kt == 7),
                        )
                    for kt in range(8):
                        nc.tensor.matmul(
                            pu[:],
                            wu_sb[:, kt, fsl],
                            xt_sb[:, kt, :tw],
                            start=(kt == 0),
                            stop=(kt == 7),
                        )
                    sl_sb = spool.tile([128, tw], BF16, tag="silu")
                    nc.scalar.activation(
                        sl_sb[:], ph[:], mybir.ActivationFunctionType.Silu
                    )
                    nc.vector.tensor_mul(ht_sb[:, ft, :], sl_sb[:], pu[:])

                # Y = (h @ Wd) * c
                for tt in range(tw // 128):
                    gi = start // 128 + tt
                    tsl = slice(tt * 128, (tt + 1) * 128)
                    for dt in range(2):
                        dsl = slice(dt * 512, (dt + 1) * 512)
                        py = psumy.tile([128, 512], F32, tag="py")
                        for kt in range(16):
                            nc.tensor.matmul(
                                py[:],
                                ht_sb[:, kt, tsl],
                                wd_sb[:, kt, dsl],
                                start=(kt == 0),
                                stop=(kt == 15),
                            )
                        o_sb = opool.tile([128, 512], F32, tag="o")
                        nc.vector.tensor_scalar_mul(
                            o_sb[:], py[:], cw_sb[:, gi : gi + 1]
                        )
                        nc.sync.dma_start(y[gi, dt], o_sb[:])

            if repeat == 1:
                body_all(first=True)
            else:
                body_all(first=True)
                with tc.For_i(0, repeat - 1, 1):
                    body_all()
    nc.compile()
    return nc


def _route_host(xf: np.ndarray, Wr: np.ndarray):
    """fp32 router replica: softmax, top-2, renorm, aux loss."""
    T = xf.shape[0]
    logits = xf @ Wr  # [T, E] fp32
    m = logits.max(axis=-1, keepdims=True)
    ex = np.exp(logits - m)
    sex = ex.sum(axis=-1, keepdims=True)
    probs = ex / sex

    ti1 = probs.argmax(axis=-1)
    p1 = probs[np.arange(T), ti1]
    pm = probs.copy()
    pm[np.arange(T), ti1] = -np.inf
    ti2 = pm.argmax(axis=-1)
    p2 = probs[np.arange(T), ti2]
    denom = p1 + p2
    c1 = p1 / denom
    c2 = p2 / denom

    counts = np.bincount(ti1, minlength=E) + np.bincount(ti2, minlength=E)
    f_i = counts.astype(np.float32) / np.float32(T * TOP_K)
    P_i = probs.mean(axis=0)
    load_balance = np.float32(E) * np.float32(np.sum(f_i * P_i))
    lse = (m[:, 0] + np.log(sex[:, 0])).astype(np.float32)
    z_loss = np.float32(np.mean(lse**2))
    aux = np.float32(LB_WEIGHT) * load_balance + np.float32(Z_WEIGHT) * z_loss
    return ti1, ti2, c1.astype(np.float32), c2.astype(np.float32), np.float32(aux)


def _prep_in_maps(xf, Wg, Wu, Wd, idxs, cs, C):
    """Pack per-expert inputs into the contiguous-slab device layouts."""
    NT = C // 128
    chunks = _chunks_for(C)
    NCH = len(chunks)
    in_maps = []
    for e in range(E):
        sel = idxs[e]
        n = len(sel)
        xe = np.zeros((C, D), dtype=np.float32)
        xe[:n] = xf[sel]
        xte = np.ascontiguousarray(xe.T).astype(ml_dtypes.bfloat16)
        arr = xte.reshape(8, 128, C)
        xt_packed = np.zeros((NCH, 8, 128, 512), dtype=ml_dtypes.bfloat16)
        for ci, (s, tw) in enumerate(chunks):
            xt_packed[ci, :, :, :tw] = arr[:, :, s : s + tw]
        ce = np.zeros(C, dtype=np.float32)
        ce[:n] = cs[e]
        in_maps.append(
            {
                "xt": xt_packed,
                "wg": np.ascontiguousarray(
                    Wg[e]
                    .astype(ml_dtypes.bfloat16)
                    .reshape(8, 128, 4, 512)
                    .transpose(2, 0, 1, 3)
                ),
                "wu": np.ascontiguousarray(
                    Wu[e]
                    .astype(ml_dtypes.bfloat16)
                    .reshape(8, 128, 4, 512)
                    .transpose(2, 0, 1, 3)
                ),
                "wd": Wd[e].astype(ml_dtypes.bfloat16).reshape(16, 128, D),
                "cw": np.ascontiguousarray(ce.reshape(NT, 128).T),
            }
        )
    return in_maps


_EXEC_CACHE = {}


def _get_executor(C):
    """Build (once per C) the bass module + jitted PJRT executor.

    Mirrors concourse.bass2jax.run_bass_via_pjrt's multi-core path, but
    caches the jitted callable so repeated kernel() calls don't re-trace.
    """
    if C in _EXEC_CACHE:
        return _EXEC_CACHE[C]

    import jax
    from jax.sharding import Mesh, PartitionSpec
    from jax.experimental.shard_map import shard_map
    from concourse.bass2jax import (
        _bass_exec_p,
        install_neuronx_cc_hook,
        partition_id_tensor,
    )

    if C not in _NC_CACHE:
        _NC_CACHE[C] = _build_nc(C)
    nc = _NC_CACHE[C]

    install_neuronx_cc_hook()
    partition_name = nc.partition_id_tensor.name if nc.partition_id_tensor else None
    in_names, out_names, out_avals, out_shapes = [], [], [], []
    for alloc in nc.m.functions[0].allocations:
        if not isinstance(alloc, mybir.MemoryLocationSet):
            continue
        name = alloc.memorylocations[0].name
        if alloc.kind == "ExternalInput":
            if name != partition_name:
                in_names.append(name)
        elif alloc.kind == "ExternalOutput":
            out_names.append(name)
            shape = tuple(alloc.tensor_shape)
            dtype = mybir.dt.np(alloc.dtype)
            out_avals.append(jax.core.ShapedArray(shape, dtype))
            out_shapes.append((shape, dtype))
    n_params = len(in_names)
    all_in_names = list(in_names) + list(out_names)
    if partition_name is not None:
        all_in_names.append(partition_name)

    def _body(*args):
        operands = list(args)
        if partition_name is not None:
            operands.append(partition_id_tensor())
        outs = _bass_exec_p.bind(
            *operands,
            out_avals=tuple(out_avals),
            in_names=tuple(all_in_names),
            out_names=tuple(out_names),
            lowering_input_output_aliases=(),
            sim_require_finite=True,
            sim_require_nnan=True,
            nc=nc,
        )
        return tuple(outs)

    devices = jax.devices()[:N_CORES]
    mesh = Mesh(np.asarray(devices), ("core",))
    n_outs = len(out_names)
    sharded = jax.jit(
        shard_map(
            _body,
            mesh=mesh,
            in_specs=(PartitionSpec("core"),) * (n_params + n_outs),
            out_specs=(PartitionSpec("core"),) * n_outs,
            check_rep=False,
        ),
        donate_argnums=tuple(range(n_params, n_params + n_outs)),
        keep_unused=True,
    )

    def run(in_maps):
        concat_in = [
            np.concatenate(
                [np.asarray(in_maps[c][nm]) for c in range(N_CORES)], axis=0
            )
            for nm in in_names
        ]
        concat_zeros = [
            np.zeros((N_CORES * s[0], *s[1:]), dt) for s, dt in out_shapes
        ]
        out_arrs = sharded(*concat_in, *concat_zeros)
        return [
            {
                nm: np.asarray(out_arrs[i]).reshape(
                    N_CORES, *out_shapes[i][0]
                )[c]
                for i, nm in enumerate(out_names)
            }
            for c in range(N_CORES)
        ]

    _EXEC_CACHE[C] = run
    return run


def kernel(x, Wr, Wg, Wu, Wd):
    x = np.asarray(x, dtype=np.float32)
    Wr = np.asarray(Wr, dtype=np.float32)
    Wg = np.asarray(Wg, dtype=np.float32)
    Wu = np.asarray(Wu, dtype=np.float32)
    Wd = np.asarray(Wd, dtype=np.float32)

    xf = x.reshape(-1, D)
    T = xf.shape[0]
    ti1, ti2, c1, c2, aux = _route_host(xf, Wr)

    # tokens per expert ("all-to-all" on host)
    idxs, cs = [], []
    for e in range(E):
        sel = np.where((ti1 == e) | (ti2 == e))[0]
        ce = np.where(ti1[sel] == e, c1[sel], c2[sel])
        idxs.append(sel)
        cs.append(ce.astype(np.float32))

    C = max(128, -(-max(len(s) for s in idxs) // 128) * 128)

    run = _get_executor(C)
    in_maps = _prep_in_maps(xf, Wg, Wu, Wd, idxs, cs, C)
    results = run(in_maps)

    out = np.zeros((T, D), dtype=np.float32)
    for e in range(E):
        sel = idxs[e]
        n = len(sel)
        # y: [NT, 2, 128, 512] -> [C, D]
        ye = results[e]["y"].transpose(0, 2, 1, 3).reshape(C, D)
        out[sel] += ye[:n]

    return out.reshape(B, S, D), aux


# revision 25
# speedup vs baseline: 1.0050x; 1.0050x over previous
"""MoE FFN (top-2, E=8) Trainium2 kernel.

Strategy (expert parallelism, per sharding hint):
  - Router (tiny: [T,D]@[D,E] + softmax/top2) computed on host in fp32,
    replicated logic; aux losses also on host (negligible FLOPs).
  - Tokens are "all-to-all"-ed on host: each of the 8 cores owns one
    expert and receives exactly the tokens routed to it (padded to a
    shared capacity C so the SPMD NEFF has static shapes).
  - Each core runs the dense SwiGLU FFN for its expert in bf16:
        h = silu(X @ Wg) * (X @ Wu);  Y = (h @ Wd) * combine[:, None]
  - Host scatter-adds the two expert contributions per token.

Device layouts (per core; all DMA slabs contiguous):
  xt: [NCH, 8, 128, 512] bf16  -- X^T per (chunk, d-k-tile), 512-token slabs
  wg: [4, 8, 128, 512]   bf16  -- Wg[e] per (f-group, d-k-tile)
  wu: [4, 8, 128, 512]   bf16
  wd: [16, 128, 1024]    bf16  -- Wd[e] per f-k-tile
  cw: [128, NT]          f32   -- combine weights, token-tile columns
  y:  [NT, 2, 128, 512]  f32   -- output slabs (already combine-scaled)

Matmuls (out = lhsT.T @ rhs, contraction on partitions):
  H^T tile [128f, tw] = sum_kt Wg[kt, :, fsl].T @ X^T[kt, :, tslice]
  Y  tile [128t, 512d] = sum_kt hT[kt][:, tslice].T @ Wd[kt, :, dslice]
so no on-device transposes are needed anywhere.
"""

import numpy as np
import ml_dtypes

import concourse.bacc as bacc
import concourse.mybir as mybir
from concourse import tile

B, S, D, F, E = 4, 2048, 1024, 2048, 8
TOP_K = 2
LB_WEIGHT = 0.01
Z_WEIGHT = 0.001
N_CORES = 8

BF16 = mybir.dt.bfloat16
F32 = mybir.dt.float32

_NC_CACHE = {}


def _chunks_for(C):
    r = C % 512
    widths = [512] * (C // 512) + ([r] if r else [])
    chunks = []
    start = 0
    for tw in widths:
        chunks.append((start, tw))
        start += tw
    return chunks


def _build_nc(C: int, repeat: int = 1):
    """Build the per-core Bass module for capacity C (multiple of 128).

    repeat>1 re-runs the compute loop (bench-only, for differential timing).
    """
    NT = C // 128
    chunks = _chunks_for(C)
    NCH = len(chunks)

    nc = bacc.Bacc(
        "TRN2", target_bir_lowering=False, debug=False, num_devices=N_CORES
    )
    xt = nc.dram_tensor("xt", [NCH, 8, 128, 512], BF16, kind="ExternalInput")
    wg = nc.dram_tensor("wg", [4, 8, 128, 512], BF16, kind="ExternalInput")
    wu = nc.dram_tensor("wu", [4, 8, 128, 512], BF16, kind="ExternalInput")
    wd = nc.dram_tensor("wd", [16, 128, 1024], BF16, kind="ExternalInput")
    cw = nc.dram_tensor("cw", [128, NT], F32, kind="ExternalInput")
    y = nc.dram_tensor("y", [NT, 2, 128, 512], F32, kind="ExternalOutput")

    with tile.TileContext(nc) as tc:
        with (
            tc.tile_pool(name="wpool", bufs=1) as wpool,
            tc.tile_pool(name="xpool", bufs=4) as xpool,
            tc.tile_pool(name="hpool", bufs=3) as hpool,
            tc.tile_pool(name="spool", bufs=4) as spool,
            tc.tile_pool(name="opool", bufs=4) as opool,
            tc.tile_pool(name="psum", bufs=3, space="PSUM") as psum,
            tc.tile_pool(name="psumy", bufs=2, space="PSUM") as psumy,
        ):
            # resident weights + combine weights
            wg_sb = wpool.tile([128, 8, F], BF16, tag="wg")
            wu_sb = wpool.tile([128, 8, F], BF16, tag="wu")
            wd_sb = wpool.tile([128, 16, D], BF16, tag="wd")
            cw_sb = wpool.tile([128, NT], F32, tag="cw")

            # chunk-0 activations first (PE's first matmul needs them), then
            # Wg/Wu streamed in 512-wide f-groups so H(ft=0) starts after
            # ~2MB of DMA instead of the full 12MB weight load.
            xt0_sb = xpool.tile([128, 8, 512], BF16, tag="xt")
            for kt in range(8):
                nc.sync.dma_start(xt0_sb[:, kt, :], xt[0, kt])
            for fg in range(4):
                fgs = slice(fg * 512, (fg + 1) * 512)
                for kt in range(8):
                    nc.sync.dma_start(wg_sb[:, kt, fgs], wg[fg, kt])
                for kt in range(8):
                    nc.sync.dma_start(wu_sb[:, kt, fgs], wu[fg, kt])
            for kt in range(16):
                nc.sync.dma_start(wd_sb[:, kt, :], wd[kt])
            nc.sync.dma_start(cw_sb[:], cw[:])

            def body_all(first=False):
                for ci, (start, tw) in enumerate(chunks):
                    body_chunk(
                        ci, start, tw, xt0_sb if (first and ci == 0) else None
                    )

            def body_chunk(ci, start, tw, preloaded=None):
                if preloaded is not None:
                    xt_sb = preloaded
                else:
                    # load X^T chunk: 8 k-tiles of [128, tw]
                    xt_sb = xpool.tile([128, 8, 512], BF16, tag="xt")
                    for kt in range(8):
                        nc.sync.dma_start(xt_sb[:, kt, :tw], xt[ci, kt][:, :tw])

                # H^T/U^T -> silu(H)*U, 16 f-tiles of [128, tw]
                ht_sb = hpool.tile([128, 16, tw], BF16, tag="ht")
                for ft in range(16):
                    ph = psum.tile([128, tw], F32, tag="ph")
                    pu = psum.tile([128, tw], F32, tag="pu")
                    fsl = slice(ft * 128, (ft + 1) * 128)
                    for kt in range(8):
                        nc.tensor.matmul(
                            ph[:],
                            wg_sb[:, kt, fsl],
                            xt_sb[:, kt, :tw],
                            start=(kt == 0),
                            stop=(kt == 7),
                        )
                    for kt in range(8):
                        nc.tensor.matmul(
                            pu[:],
                            wu_sb[:, kt, fsl],
                            xt_sb[:, kt, :tw],
                            start=(kt == 0),
                            stop=(kt == 7),
                        )
                    sl_sb = spool.tile([128, tw], BF16, tag="silu")
                    nc.scalar.activation(
                        sl_sb[:], ph[:], mybir.ActivationFunctionType.Silu
                    )
                    nc.vector.tensor_mul(ht_sb[:, ft, :], sl_sb[:], pu[:])

                # Y = (h @ Wd) * c
                for tt in range(tw // 128):
                    gi = start // 128 + tt
                    tsl = slice(tt * 128, (tt + 1) * 128)
                    for dt in range(2):
                        dsl = slice(dt * 512, (dt + 1) * 512)
                        py = psumy.tile([128, 512], F32, tag="py")
                        for kt in range(16):
                            nc.tensor.matmul(
                                py[:],
                                ht_sb[:, kt, tsl],
                                wd_sb[:, kt, dsl],
                                start=(kt == 0),
                                stop=(kt == 15),
                            )
                        o_sb = opool.tile([128, 512], F32, tag="o")
                        nc.vector.tensor_scalar_mul(
                            o_sb[:], py[:], cw_sb[:, gi : gi + 1]
                        )
                        nc.sync.dma_start(y[gi, dt], o_sb[:])

            if repeat == 1:
                body_all(first=True)
            else:
                body_all(first=True)
                with tc.For_i(0, repeat - 1, 1):
                    body_all()
    nc.compile()
    return nc


def _route_host(xf: np.ndarray, Wr: np.ndarray):
    """fp32 router replica: softmax, top-2, renorm, aux loss."""
    T = xf.shape[0]
    logits = xf @ Wr  # [T, E] fp32
    m = logits.max(axis=-1, keepdims=True)
    ex = np.exp(logits - m)
    sex = ex.sum(axis=-1, keepdims=True)
    probs = ex / sex

    ti1 = probs.argmax(axis=-1)
    p1 = probs[np.arange(T), ti1]
    pm = probs.copy()
    pm[np.arange(T), ti1] = -np.inf
    ti2 = pm.argmax(axis=-1)
    p2 = probs[np.arange(T), ti2]
    denom = p1 + p2
    c1 = p1 / denom
    c2 = p2 / denom

    counts = np.bincount(ti1, minlength=E) + np.bincount(ti2, minlength=E)
    f_i = counts.astype(np.float32) / np.float32(T * TOP_K)
    P_i = probs.mean(axis=0)
    load_balance = np.float32(E) * np.float32(np.sum(f_i * P_i))
    lse = (m[:, 0] + np.log(sex[:, 0])).astype(np.float32)
    z_loss = np.float32(np.mean(lse**2))
    aux = np.float32(LB_WEIGHT) * load_balance + np.float32(Z_WEIGHT) * z_loss
    return ti1, ti2, c1.astype(np.float32), c2.astype(np.float32), np.float32(aux)


def _prep_in_maps(xf, Wg, Wu, Wd, idxs, cs, C):
    """Pack per-expert inputs into the contiguous-slab device layouts."""
    NT = C // 128
    chunks = _chunks_for(C)
    NCH = len(chunks)
    in_maps = []
    for e in range(E):
        sel = idxs[e]
        n = len(sel)
        xe = np.zeros((C, D), dtype=np.float32)
        xe[:n] = xf[sel]
        xte = np.ascontiguousarray(xe.T).astype(ml_dtypes.bfloat16)
        arr = xte.reshape(8, 128, C)
        xt_packed = np.zeros((NCH, 8, 128, 512), dtype=ml_dtypes.bfloat16)
        for ci, (s, tw) in enumerate(chunks):
            xt_packed[ci, :, :, :tw] = arr[:, :, s : s + tw]
        ce = np.zeros(C, dtype=np.float32)
        ce[:n] = cs[e]
        in_maps.append(
            {
                "xt": xt_packed,
                "wg": np.ascontiguousarray(
                    Wg[e]
                    .astype(ml_dtypes.bfloat16)
                    .reshape(8, 128, 4, 512)
                    .transpose(2, 0, 1, 3)
                ),
                "wu": np.ascontiguousarray(
                    Wu[e]
                    .astype(ml_dtypes.bfloat16)
                    .reshape(8, 128, 4, 512)
                    .transpose(2, 0, 1, 3)
                ),
                "wd": Wd[e].astype(ml_dtypes.bfloat16).reshape(16, 128, D),
                "cw": np.ascontiguousarray(ce.reshape(NT, 128).T),
            }
        )
    return in_maps


_EXEC_CACHE = {}


def _get_executor(C):
    """Build (once per C) the bass module + jitted PJRT executor.

    Mirrors concourse.bass2jax.run_bass_via_pjrt's multi-core path, but
    caches the jitted callable so repeated kernel() calls don't re-trace.
    """
    if C in _EXEC_CACHE:
        return _EXEC_CACHE[C]

    import jax
    from jax.sharding import Mesh, PartitionSpec
    from jax.experimental.shard_map import shard_map
    from concourse.bass2jax import (
        _bass_exec_p,
        install_neuronx_cc_hook,
        partition_id_tensor,
    )

    if C not in _NC_CACHE:
        _NC_CACHE[C] = _build_nc(C)
    nc = _NC_CACHE[C]

    install_neuronx_cc_hook()
    partition_name = nc.partition_id_tensor.name if nc.partition_id_tensor else None
    in_names, out_names, out_avals, out_shapes = [], [], [], []
    for alloc in nc.m.functions[0].allocations:
        if not isinstance(alloc, mybir.MemoryLocationSet):
            continue
        name = alloc.memorylocations[0].name
        if alloc.kind == "ExternalInput":
            if name != partition_name:
                in_names.append(name)
        elif alloc.kind == "ExternalOutput":
            out_names.append(name)
            shape = tuple(alloc.tensor_shape)
            dtype = mybir.dt.np(alloc.dtype)
            out_avals.append(jax.core.ShapedArray(shape, dtype))
            out_shapes.append((shape, dtype))
    n_params = len(in_names)
    all_in_names = list(in_names) + list(out_names)
    if partition_name is not None:
        all_in_names.append(partition_name)

    def _body(*args):
        operands = list(args)
        if partition_name is not None:
            operands.append(partition_id_tensor())
        outs = _bass_exec_p.bind(
            *operands,
            out_avals=tuple(out_avals),
            in_names=tuple(all_in_names),
            out_names=tuple(out_names),
            lowering_input_output_aliases=(),
            sim_require_finite=True,
            sim_require_nnan=True,
            nc=nc,
        )
        return tuple(outs)

    devices = jax.devices()[:N_CORES]
    mesh = Mesh(np.asarray(devices), ("core",))
    n_outs = len(out_names)
    sharded = jax.jit(
        shard_map(
            _body,
            mesh=mesh,
            in_specs=(PartitionSpec("core"),) * (n_params + n_outs),
            out_specs=(PartitionSpec("core"),) * n_outs,
            check_rep=False,
        ),
        donate_argnums=tuple(range(n_params, n_params + n_outs)),
        keep_unused=True,
    )

    def run(in_maps):
        concat_in = [
            np.concatenate(
                [np.asarray(in_maps[c][nm]) for c in range(N_CORES)], axis=0
            )
            for nm in in_names
        ]
        concat_zeros = [
            np.zeros((N_CORES * s[0], *s[1:]), dt) for s, dt in out_shapes
        ]
        out_arrs = sharded(*concat_in, *concat_zeros)
        return [
            {
                nm: np.asarray(out_arrs[i]).reshape(
                    N_CORES, *out_shapes[i][0]
                )[c]
                for i, nm in enumerate(out_names)
            }
            for c in range(N_CORES)
        ]

    _EXEC_CACHE[C] = run
    return run


def kernel(x, Wr, Wg, Wu, Wd):
    x = np.asarray(x, dtype=np.float32)
    Wr = np.asarray(Wr, dtype=np.float32)
    Wg = np.asarray(Wg, dtype=np.float32)
    Wu = np.asarray(Wu, dtype=np.float32)
    Wd = np.asarray(Wd, dtype=np.float32)

    xf = x.reshape(-1, D)
    T = xf.shape[0]
    ti1, ti2, c1, c2, aux = _route_host(xf, Wr)

    # tokens per expert ("all-to-all" on host)
    idxs, cs = [], []
    for e in range(E):
        sel = np.where((ti1 == e) | (ti2 == e))[0]
        ce = np.where(ti1[sel] == e, c1[sel], c2[sel])
        idxs.append(sel)
        cs.append(ce.astype(np.float32))

    C = max(128, -(-max(len(s) for s in idxs) // 128) * 128)

    run = _get_executor(C)
    in_maps = _prep_in_maps(xf, Wg, Wu, Wd, idxs, cs, C)
    results = run(in_maps)

    out = np.zeros((T, D), dtype=np.float32)
    for e in range(E):
        sel = idxs[e]
        n = len(sel)
        # y: [NT, 2, 128, 512] -> [C, D]
        ye = results[e]["y"].transpose(0, 2, 1, 3).reshape(C, D)
        out[sel] += ye[:n]

    return out.reshape(B, S, D), aux
